# revision 26
# baseline (speedup 1.0000x reference)
"""Trainium2 Bass kernel for AttentionSequencePoolingLayer (DIN-style attention).

Reference computation (per batch b):
    att_in = concat([q, k, q-k, q*k], -1)            (T, 4E)
    h1 = relu(att_in @ W1 + b1)                      (T, 80)
    h2 = relu(h1 @ W2 + b2)                          (T, 40)
    s  = h2 @ W3 + b3                                (T, 1)
    out = (mask * s).T @ k                           (1, E)

Key restructurings vs a naive port:

1. q is constant over T, so with W1 = [W1a; W1b; W1c; W1d] (blocks of E rows)
   the whole layer-1 collapses to ONE per-batch matmul:
       h1.T = relu( W_b.T @ k.T + C[:, b] )
       W_b  = (W1b - W1c) + q_b * W1d     (per-batch stationary, [E, 80])
       C    = (W1a + W1c).T @ Q.T + b1    (one matmul for all batches)
   W_b is generated on the otherwise-idle GpSimd (Pool) engine; the per-batch
   bias C[:, b] rides in on the scalar-engine relu evacuation.

2. All PE transposes of the keys are replaced by XBAR DMA transposes:
   keys are passed from the host as bf16 padded to T_PAD=208 rows, and each
   supergroup's k.T tile arrives via ONE dma_start(transpose=True) straight
   from DRAM.  Natural-layout bf16 keys (for the pooling stationary) are
   loaded by plain strided DMA.  The query / masks / output transposes are
   likewise DMA'd or host-side.

3. Everything the PE touches is bf16 (1 cycle/row streams, 2 rows/cycle
   weight loads); psum accumulation stays fp32.

Scores are produced on T-partitions by small transposed matmuls (lhsT = h2.T
column slices, rhs = padded W3), masked during evacuation with DMA-transposed
masks (b3 rides in as b3*mask), then pooling accumulates poolT[:, b] =
keys_nat.T @ masked_scores as two-column matmuls.  The pooled result is kept
E-on-partitions and written transposed; the host untransposes.

Sharding: pure data parallel, batch dim split across 8 NeuronCores
(256 batches per core), 16-batch supergroups.
"""

from contextlib import ExitStack

import numpy as np
import ml_dtypes

import concourse.bass as bass
import concourse.bacc as bacc
import concourse.tile as tile
from concourse import mybir
from concourse.bass_utils import run_bass_kernel_spmd


B, T, E = 2048, 200, 128
H1, H2 = 80, 40
N_CORES = 8
B_CORE = B // N_CORES   # 256
T_PAD = 208             # keys padded on host: %16==0 for the XBAR transpose
M_PAD = 256             # masks padded on host: %128==0 for the XBAR transpose
SG = 16                 # batches per supergroup
GRP = 2                 # batches per L2 matmul group (N = GRP*T = 400)
TA, TB = 128, 80        # key-row split across partitions (TA+TB = T_PAD)
TBV = T - TA            # valid rows in the TB half (72)

F32 = mybir.dt.float32
BF16 = mybir.dt.bfloat16
AF = mybir.ActivationFunctionType
OP = mybir.AluOpType


def build(b_core=B_CORE):
    nc = bacc.Bacc("TRN2", target_bir_lowering=False, debug=False,
                   num_devices=N_CORES)
    q_d = nc.dram_tensor("query16", [b_core, E], BF16, kind="ExternalInput")
    k_d = nc.dram_tensor("keys16", [b_core, T_PAD, E], BF16,
                         kind="ExternalInput")
    m_d = nc.dram_tensor("masks16", [b_core, M_PAD], BF16,
                         kind="ExternalInput")
    w1_d = nc.dram_tensor("W1", [4 * E, H1], F32, kind="ExternalInput")
    b1_d = nc.dram_tensor("b1", [H1], F32, kind="ExternalInput")
    w2_d = nc.dram_tensor("W2", [H1, H2], F32, kind="ExternalInput")
    b2_d = nc.dram_tensor("b2", [H2], F32, kind="ExternalInput")
    w3_d = nc.dram_tensor("W3", [H2, 1], F32, kind="ExternalInput")
    b3_d = nc.dram_tensor("b3", [1], F32, kind="ExternalInput")
    out_d = nc.dram_tensor("outT", [E, b_core], F32, kind="ExternalOutput")

    with tile.TileContext(nc) as tc:
        _body(tc, nc, q_d, k_d, m_d, w1_d, b1_d, w2_d, b2_d, w3_d, b3_d,
              out_d, b_core)
    nc.compile()
    return nc


def _body(tc, nc, q_d, k_d, m_d, w1_d, b1_d, w2_d, b2_d, w3_d, b3_d, out_d,
          b_core):
    ctx = ExitStack()
    with ctx:
        consts = ctx.enter_context(tc.tile_pool(name="consts", bufs=1))

        # ---- weights ----
        w1s = consts.tile([128, 4, H1], F32)
        nc.sync.dma_start(out=w1s, in_=w1_d.rearrange("(f p) c -> p f c", p=128))
        # A = W1b - W1c (bf16), D = W1d (bf16), replicated SG times for the
        # per-supergroup W_all generation on gpsimd.
        a16 = consts.tile([128, SG, H1], BF16)
        nc.vector.tensor_tensor(out=a16[:, 0, :], in0=w1s[:, 1, :],
                                in1=w1s[:, 2, :], op=OP.subtract)
        d16 = consts.tile([128, H1], BF16)
        nc.vector.tensor_copy(out=d16, in_=w1s[:, 3, :])
        for j in range(1, SG):
            nc.vector.tensor_copy(out=a16[:, j, :], in_=a16[:, 0, :])
        w1ac = consts.tile([128, H1], BF16)
        nc.vector.tensor_tensor(out=w1ac, in0=w1s[:, 0, :], in1=w1s[:, 2, :],
                                op=OP.add)
        w2f = consts.tile([H1, H2], F32)
        nc.sync.dma_start(out=w2f, in_=w2_d.ap())
        w2b = consts.tile([H1, H2], BF16)
        nc.vector.tensor_copy(out=w2b, in_=w2f)
        w3f = consts.tile([H2, 1], F32)
        nc.sync.dma_start(out=w3f, in_=w3_d.ap())
        w3pb = consts.tile([H2, 2], BF16)
        nc.vector.memset(w3pb, 0.0)
        nc.vector.tensor_copy(out=w3pb[:, 0:1], in_=w3f)
        b1 = consts.tile([H1, 1], F32)
        nc.sync.dma_start(
            out=b1, in_=bass.AP(tensor=b1_d.ap().tensor, offset=0,
                                ap=[[1, H1], [1, 1]]))
        b2 = consts.tile([H2, 1], F32)
        nc.sync.dma_start(
            out=b2, in_=bass.AP(tensor=b2_d.ap().tensor, offset=0,
                                ap=[[1, H2], [1, 1]]))
        b3bc = consts.tile([128, 1], F32)
        nc.sync.dma_start(
            out=b3bc, in_=bass.AP(tensor=b3_d.ap().tensor, offset=0,
                                  ap=[[0, 128], [1, 1]]))
        zerob = consts.tile([128, 2 * SG], BF16)
        nc.vector.memset(zerob, 0.0)

        # ---- Q.T via XBAR DMA transpose: (E, batch) bf16 ----
        qt = consts.tile([128, b_core], BF16)
        nc.sync.dma_start(out=qt, in_=q_d.ap(), transpose=True)
        qtf = consts.tile([128, b_core], F32)
        nc.vector.tensor_copy(out=qtf, in_=qt)

        # ---- masks, DMA-transposed to (t, batch); plus b3 * mask ----
        mt0 = consts.tile([TA, b_core], BF16)
        nc.sync.dma_start(out=mt0, in_=m_d[:, 0:128], transpose=True)
        mt1 = consts.tile([128, b_core], BF16)
        nc.sync.dma_start(out=mt1, in_=m_d[:, 128:256], transpose=True)
        b3m0 = consts.tile([TA, b_core], BF16)
        nc.vector.tensor_scalar_mul(b3m0, mt0, b3bc[0:TA, :])
        b3m1 = consts.tile([TBV, b_core], BF16)
        nc.vector.tensor_scalar_mul(b3m1, mt1[0:TBV, :], b3bc[0:TBV, :])

        # pooled output, transposed: (E, batch)
        poolt_sb = consts.tile([128, b_core], F32)

        # ---- main pipeline pools ----
        kstA = ctx.enter_context(tc.tile_pool(name="kstA", bufs=3))
        kstB = ctx.enter_context(tc.tile_pool(name="kstB", bufs=3))
        ktp = ctx.enter_context(tc.tile_pool(name="ktp", bufs=3))
        wal = ctx.enter_context(tc.tile_pool(name="wal", bufs=4))
        work = ctx.enter_context(tc.tile_pool(name="work", bufs=18))
        work2 = ctx.enter_context(tc.tile_pool(name="work2", bufs=4))
        stp = ctx.enter_context(tc.tile_pool(name="stp", bufs=3))
        h1_ps = ctx.enter_context(tc.tile_pool(name="h1_ps", bufs=3, space="PSUM"))
        h2_ps = ctx.enter_context(tc.tile_pool(name="h2_ps", bufs=2, space="PSUM"))
        sm_ps = ctx.enter_context(tc.tile_pool(name="sm_ps", bufs=3, space="PSUM"))

        # ---- C = (W1a+W1c).T @ Q.T + b1 (borrows an h1 psum bank) ----
        cps = h1_ps.tile([H1, GRP * T], F32, tag="h1p")
        nc.tensor.matmul(cps[:, :b_core], lhsT=w1ac, rhs=qt,
                         start=True, stop=True)
        csb = consts.tile([H1, b_core], F32)
        nc.scalar.activation(out=csb, in_=cps[:, :b_core], func=AF.Identity,
                             bias=b1)

        n_sg = b_core // SG
        NCOL = GRP * T  # 400

        def issue_dma(sg):
            b0 = sg * SG
            nb = min(SG, b_core - b0)
            # k.T for the whole supergroup: ONE XBAR DMA transpose from DRAM
            kt = ktp.tile([128, SG * T_PAD], BF16, tag="kt")
            nc.sync.dma_start(
                out=kt[:, :nb * T_PAD],
                in_=k_d[b0:b0 + nb].rearrange("b t e -> (b t) e"),
                transpose=True)
            # natural keys (t on partitions) for the pooling stationary
            tA = kstA.tile([TA, SG, E], BF16, tag="tA")
            nc.sync.dma_start(
                out=tA[:, :nb, :],
                in_=k_d[b0:b0 + nb, 0:TA, :].rearrange("b t e -> t b e"))
            tB = kstB.tile([TB, SG, E], BF16, tag="tB")
            nc.sync.dma_start(
                out=tB[:, :nb, :],
                in_=k_d[b0:b0 + nb, TA:T_PAD, :].rearrange("b t e -> t b e"))
            return kt, tA, tB

        def issue_wall(sg):
            # per-batch L1 stationaries on vector: W_all[:, j, :] = A + q_j*D
            # (broadcast APs: D repeats over the batch axis, q over the H1
            # axis — stride-0 dims)
            b0 = sg * SG
            nb = min(SG, b_core - b0)
            wall = wal.tile([128, SG, H1], BF16, tag="wall")
            qd = wal.tile([128, SG, H1], BF16, tag="qd")
            d_bc = bass.AP(tensor=d16.tensor, offset=d16.offset,
                           ap=[list(d16.ap[0]), [0, nb], [1, H1]])
            q_ap = qt[:, b0:b0 + nb]
            q_bc = bass.AP(tensor=q_ap.tensor, offset=q_ap.offset,
                           ap=[list(q_ap.ap[0]), [q_ap.ap[1][0], nb], [0, H1]])
            nc.gpsimd.tensor_tensor(out=qd[:, :nb, :], in0=d_bc, in1=q_bc,
                                     op=OP.mult)
            nc.gpsimd.tensor_tensor(out=wall[:, :nb, :], in0=qd[:, :nb, :],
                                    in1=a16[:, :nb, :], op=OP.add)
            return wall

        def stage_l1(sg, kt, wall):
            # L1 (one matmul per batch, per-batch stationary W_b) + scalar
            # relu/bias evacuation, one supergroup AHEAD of L2.
            b0 = sg * SG
            nb = min(SG, b_core - b0)
            h1s = []
            for g in range(nb // GRP):
                h1p = h1_ps.tile([H1, NCOL], F32, tag="h1p")
                for j in range(GRP):
                    lb = GRP * g + j
                    nc.tensor.matmul(
                        h1p[:, j * T:(j + 1) * T],
                        lhsT=wall[:, lb, :],
                        rhs=kt[:, lb * T_PAD:lb * T_PAD + T],
                        start=True, stop=True)
                h1 = work.tile([H1, NCOL], BF16, tag="h1")
                for j in range(GRP):
                    gb = b0 + GRP * g + j
                    if g % 2 == 0:
                        nc.scalar.activation(
                            out=h1[:, j * T:(j + 1) * T],
                            in_=h1p[:, j * T:(j + 1) * T],
                            func=AF.Relu, bias=csb[:, gb:gb + 1])
                    else:
                        nc.vector.tensor_scalar(
                            out=h1[:, j * T:(j + 1) * T],
                            in0=h1p[:, j * T:(j + 1) * T],
                            scalar1=csb[:, gb:gb + 1], scalar2=0.0,
                            op0=OP.add, op1=OP.max)
                h1s.append(h1)
            return h1s

        def stage_l2(sg, h1s):
            # L2 matmuls + vector relu/b2 evac + score minis
            b0 = sg * SG
            nb = min(SG, b_core - b0)
            smbig = sm_ps.tile([128, 8 * SG], F32, tag="smbig")
            stA_ps = smbig[:, 0:2 * SG]
            stB_ps = smbig[0:TBV, 2 * SG:4 * SG]
            h2s = []
            for g in range(nb // GRP):
                h2p = h2_ps.tile([H2, NCOL], F32, tag="h2p")
                nc.tensor.matmul(h2p, lhsT=w2b, rhs=h1s[g],
                                 start=True, stop=True)
                h2 = work2.tile([H2, NCOL], BF16, tag="h2")
                nc.scalar.activation(out=h2, in_=h2p, func=AF.Relu, bias=b2)
                h2s.append(h2)
            return smbig, h2s

        def stage_minis(sg, smh):
            b0 = sg * SG
            nb = min(SG, b_core - b0)
            smbig, h2s = smh
            stA_ps = smbig[:, 0:2 * SG]
            stB_ps = smbig[0:TBV, 2 * SG:4 * SG]
            for g in range(nb // GRP):
                h2 = h2s[g]
                for j in range(GRP):
                    c = j * T
                    o = 2 * (GRP * g + j)
                    nc.tensor.matmul(stA_ps[:, o:o + 2],
                                     lhsT=h2[:, c:c + TA], rhs=w3pb,
                                     start=True, stop=True)
                    nc.tensor.matmul(stB_ps[:, o:o + 2],
                                     lhsT=h2[:, c + TA:c + T], rhs=w3pb,
                                     start=True, stop=True)
            return smbig

        def stage_mask(sg, smbig):
            # masked scores on vector: sT_m = sT * m + b3 * m
            b0 = sg * SG
            nb = min(SG, b_core - b0)
            stA_ps = smbig[:, 0:2 * SG]
            stB_ps = smbig[0:TBV, 2 * SG:4 * SG]
            stA_s = stA_ps.rearrange("p (b two) -> p b two", two=2)[:, :, 0]
            stB_s = stB_ps.rearrange("p (b two) -> p b two", two=2)[:, :, 0]
            stA = stp.tile([TA, 2 * SG], BF16, tag="stA")
            nc.gpsimd.tensor_copy(out=stA, in_=zerob[:TA, :])
            stAv = stA.rearrange("p (b two) -> p b two", two=2)[:, :, 0]
            nc.vector.tensor_tensor(out=stAv[:, :nb], in0=stA_s[:, :nb],
                                    in1=mt0[:, b0:b0 + nb], op=OP.mult)
            nc.vector.tensor_tensor(out=stAv[:, :nb], in0=stAv[:, :nb],
                                    in1=b3m0[:, b0:b0 + nb], op=OP.add)
            stB = stp.tile([TB, 2 * SG], BF16, tag="stB")
            nc.gpsimd.tensor_copy(out=stB[:, :], in_=zerob[:TB, :])
            stBv = stB.rearrange("p (b two) -> p b two", two=2)[:TBV, :, 0]
            nc.vector.tensor_tensor(out=stBv[:, :nb], in0=stB_s[:, :nb],
                                    in1=mt1[0:TBV, b0:b0 + nb], op=OP.mult)
            nc.vector.tensor_tensor(out=stBv[:, :nb], in0=stBv[:, :nb],
                                    in1=b3m1[:, b0:b0 + nb], op=OP.add)
            return stA, stB

        def stage_pool(sg, tA, tB, stA, stB, smbig):
            b0 = sg * SG
            nb = min(SG, b_core - b0)
            plA = smbig[:, 4 * SG:6 * SG]
            for lb in range(nb):
                nc.tensor.matmul(plA[:, 2 * lb:2 * lb + 2],
                                 lhsT=tA[:, lb, :],
                                 rhs=stA[:, 2 * lb:2 * lb + 2],
                                 start=True, stop=False)
                nc.tensor.matmul(plA[:, 2 * lb:2 * lb + 2],
                                 lhsT=tB[:, lb, :],
                                 rhs=stB[:, 2 * lb:2 * lb + 2],
                                 start=False, stop=True)
            return smbig

        # ---- software-pipelined main loop ----
        # iteration i: DMA(i+2)/wall(i+2) issue, L1+relu1(i+1),
        # L2+minis(i), mask(i), pool(i-1), pool-evac(i-2)
        dmas = {0: issue_dma(0)}
        walls = {0: issue_wall(0)}
        if n_sg > 1:
            dmas[1] = issue_dma(1)
            walls[1] = issue_wall(1)
        h1_d = {0: stage_l1(0, dmas[0][0], walls[0])}
        sm_d = {}
        st_d = {}
        pool_d = {}
        for sg in range(n_sg):
            if sg + 2 < n_sg:
                dmas[sg + 2] = issue_dma(sg + 2)
                walls[sg + 2] = issue_wall(sg + 2)
            if sg + 1 < n_sg:
                h1_d[sg + 1] = stage_l1(sg + 1, dmas[sg + 1][0],
                                        walls.pop(sg + 1))
            smh = stage_l2(sg, h1_d.pop(sg))
            if sg - 1 >= 0:
                tA, tB = dmas[sg - 1][1], dmas[sg - 1][2]
                stA, stB = st_d.pop(sg - 1)
                pool_d[sg - 1] = stage_pool(sg - 1, tA, tB, stA, stB,
                                            sm_d.pop(sg - 1))
                dmas.pop(sg - 1)
            sm_d[sg] = stage_minis(sg, smh)
            st_d[sg] = stage_mask(sg, sm_d[sg])
            if sg - 2 >= 0:
                _evac_pool(nc, stp, poolt_sb, pool_d.pop(sg - 2), sg - 2,
                           b_core)

        # drain
        sg = n_sg - 1
        tA, tB = dmas[sg][1], dmas[sg][2]
        stA, stB = st_d.pop(sg)
        pool_d[sg] = stage_pool(sg, tA, tB, stA, stB, sm_d.pop(sg))
        if n_sg >= 2:
            _evac_pool(nc, stp, poolt_sb, pool_d.pop(n_sg - 2), n_sg - 2,
                       b_core)
        _evac_pool(nc, stp, poolt_sb, pool_d.pop(sg), sg, b_core)

        # ---- final: store pooled (E, batch); host untransposes ----
        nc.sync.dma_start(out=out_d.ap(), in_=poolt_sb)


def _evac_pool(nc, stp, poolt_sb, smbig, sg, b_core):
    b0 = sg * SG
    nb = min(SG, b_core - b0)
    plA = smbig[:, 4 * SG:6 * SG]
    plA_s = plA.rearrange("p (b two) -> p b two", two=2)[:, :, 0]
    nc.vector.tensor_copy(out=poolt_sb[:, b0:b0 + nb], in_=plA_s[:, :nb])


_NC_CACHE = {}


def _get_nc(b_core=B_CORE):
    if b_core not in _NC_CACHE:
        _NC_CACHE[b_core] = build(b_core)
    return _NC_CACHE[b_core]


def kernel(query, keys, key_masks, W1, b1, W2, b2, W3, b3, _trace=False):
    bf16 = ml_dtypes.bfloat16
    q16 = np.ascontiguousarray(
        np.asarray(query, dtype=np.float32).reshape(B, E)).astype(bf16)
    k16 = np.zeros((B, T_PAD, E), dtype=bf16)
    k16[:, :T, :] = np.asarray(keys, dtype=np.float32)
    m16 = np.zeros((B, M_PAD), dtype=bf16)
    m16[:, :T] = np.asarray(key_masks).reshape(B, T)
    nc = _get_nc()
    in_maps = []
    for c in range(N_CORES):
        sl = slice(c * B_CORE, (c + 1) * B_CORE)
        in_maps.append({
            "query16": q16[sl],
            "keys16": k16[sl],
            "masks16": m16[sl],
            "W1": np.asarray(W1, dtype=np.float32),
            "b1": np.asarray(b1, dtype=np.float32),
            "W2": np.asarray(W2, dtype=np.float32),
            "b2": np.asarray(b2, dtype=np.float32),
            "W3": np.asarray(W3, dtype=np.float32),
            "b3": np.asarray(b3, dtype=np.float32),
        })
    res = run_bass_kernel_spmd(nc, in_maps, list(range(N_CORES)), trace=_trace)
    out = np.concatenate(
        [res.results[c]["outT"].T.reshape(B_CORE, 1, E)
         for c in range(N_CORES)], axis=0)
    if _trace:
        kernel.last_exec_time_ns = res.exec_time_ns
        kernel.last_results = res
    return out.astype(np.float32)


kernel.last_exec_time_ns = None
kernel.last_results = None
